# revision 19
# baseline (speedup 1.0000x reference)
"""GCN sampling (NodeFlow) kernel for 8 TRN2 NeuronCores — v3.

Geometry (hardcoded per problem spec):
  N0=409600 nodes x 512 feats, layer0: 40960 dst x fanout 10, W1 [512,256]+relu,
  layer1: 4096 dst x fanout 10, W2 [256,64].

Strategy: shard layer-1 dst nodes across 8 cores (512 each). Each core pulls,
for each of its 5120 layer-1 edges, the 10 layer-0 feature rows of that edge's
src h-row (indices precomputed on host; h-rows deliberately duplicated per
layer-1 edge so BOTH segment-means become fixed-stride pooling, no on-device
gather for layer 1 and no cross-core communication). 1/10 mean factors are
folded into W1, W2 on the host.

The modeled bottleneck is the Pool engine's SWDGE descriptor generation for
the 400 indirect gathers (the HW DGE honors exactly one index per partition
per instruction — verified by probe; multi-index gathers return garbage for
slots >0, and the InstDMAGatherAnt custom path crashes this runtime). Per
gather: ~1040ns descriptor-gen vs 728ns DMA transfer, so ~415us Pool-bound.
Everything else is arranged to hide under it and to keep the head/tail of the
pipeline short:
  * block-0 indices are loaded as a separate tiny DMA so the first gather
    starts as early as possible; all constants load via HWDGE (SP engine),
    keeping Pool exclusively on gather descriptor generation.
  * per block of 128 edges: DVE pairwise-tree pools the 10 slots (f32),
    4 single-shot PE transposes produce h0^T, DVE downcasts PSUM->bf16,
    W1 matmul in bf16 (1 cycle/row) + ReLU per block.
  * hop-2 pooling is a per-block stride-2 pair-add (pairs never straddle
    blocks) plus a 5-block chunk tail (64 dst): 5->1 tree, W2 (bf16), store.
    The post-last-gather critical chain is one block's compute + one small
    chunk tail instead of a full-width epilogue.
"""

import sys

sys.path.insert(0, "/opt/trn_rl_repo")

from contextlib import ExitStack

import numpy as np

N0, N1, N2 = 409600, 40960, 4096
F = 10                      # fanout
IN_F, HID, NCLS = 512, 256, 64
NC_N = 8                    # cores
DST_PC = N2 // NC_N         # 512 dst nodes per core
GRP_PC = DST_PC * F         # 5120 h-rows (edges) per core
BLK = 128                   # edges per block (partition dim)
NBLK = GRP_PC // BLK        # 40 blocks
CHUNK_BLKS = 5              # blocks per output chunk (640 edges = 64 dst)
NCHUNK = NBLK // CHUNK_BLKS  # 8 output chunks
CHUNK_DST = BLK * CHUNK_BLKS // F   # 64 dst rows per chunk

_BUILT = None


NBLK_D = 38                 # slot blocks after dedup
UB = NBLK_D * BLK           # 4864 slots
NSC = 10                    # superchunks of <=512 slot-cols
SENT = 600                  # f-table sentinel: no referencing dst

def _legalize_waits(bir: bytes) -> bytes:
    """This container's walrus supports exactly ONE sync-wait per instruction.
    Split every multi-wait instruction: keep the last wait, hoist the others
    onto single-wait EventSemaphore instructions inserted just before it on
    the same engine (same semantics: engine sequencer blocks in order)."""
    import orjson

    j = orjson.loads(bir)
    n_new = 0
    for fn in j["functions"]:
        for bb in fn["blocks"]:
            insts = bb["instructions"]
            out = []
            for inst in insts:
                si = inst.get("sync_info")
                waits = si.get("on_wait") if si else None
                if waits and len(waits) > 1:
                    for w in waits[:-1]:
                        n_new += 1
                        out.append({
                            "debug": inst.get("debug", 0),
                            "engine": inst["engine"],
                            "ins": [],
                            "name": f"{inst['name']}_esw{n_new}",
                            "opcode": "EventSemaphore",
                            "outs": [],
                            "sync_info": {"on_update": [], "on_wait": [w]},
                        })
                    si["on_wait"] = [waits[-1]]
                out.append(inst)
            bb["instructions"] = out
    return orjson.dumps(j)


def _install_patch():
    import concourse.bass as bass

    if getattr(bass.Bass, "_gcn_wait_patch", False):
        return
    orig = bass.Bass.to_json_bytes

    def to_json_bytes(self, *a, **kw):
        return _legalize_waits(orig(self, *a, **kw))

    bass.Bass.to_json_bytes = to_json_bytes
    bass.Bass._gcn_wait_patch = True


def build_nc(conservative=False):
    _install_patch()
    SPLIT_B, SPLIT_D = 30, 384
    import concourse.bass as bass
    import concourse.tile as tile
    from concourse import mybir

    f32 = mybir.dt.float32
    bf16 = mybir.dt.bfloat16
    i32 = mybir.dt.int32
    nc = bass.Bass("TRN2", target_bir_lowering=False, debug=False,
                   num_devices=NC_N, num_swdge_queues=4)

    feat = nc.dram_tensor("feat", [N0, IN_F], f32, kind="ExternalInput")
    w1t_d = nc.dram_tensor("w1t", [128, 4 * HID], bf16, kind="ExternalInput")
    b1t_d = nc.dram_tensor("b1t", [128, 2], f32, kind="ExternalInput")
    w2t_d = nc.dram_tensor("w2t", [128, 2 * NCLS], bf16, kind="ExternalInput")
    identb_d = nc.dram_tensor("identb", [128, 128], f32, kind="ExternalInput")
    identb2_d = nc.dram_tensor("identb2", [128, 128], bf16, kind="ExternalInput")
    idx0_d = nc.dram_tensor("idx0", [BLK, F], i32, kind="ExternalInput")
    idxr_d = nc.dram_tensor("idxr", [BLK, (NBLK_D - 1) * F], i32,
                            kind="ExternalInput")
    ft_d = nc.dram_tensor("ft", [BLK, 2 * NBLK_D], f32, kind="ExternalInput")
    iota_d = nc.dram_tensor("iota", [128, DST_PC], f32, kind="ExternalInput")
    out = nc.dram_tensor("out", [DST_PC, NCLS], f32, kind="ExternalOutput")

    with tile.TileContext(nc) as tc, ExitStack() as ctx:
        consts = ctx.enter_context(tc.tile_pool(name="consts", bufs=1))
        gpool = ctx.enter_context(tc.tile_pool(name="gather", bufs=3))
        spool = ctx.enter_context(tc.tile_pool(name="pooled", bufs=2))
        hpool = ctx.enter_context(tc.tile_pool(name="hhT", bufs=2))
        h1pool = ctx.enter_context(tc.tile_pool(name="h1T", bufs=1))
        wpool = ctx.enter_context(tc.tile_pool(name="w2s", bufs=2))
        apool = ctx.enter_context(tc.tile_pool(name="amat", bufs=2))
        opool = ctx.enter_context(tc.tile_pool(name="outs", bufs=2))
        ps_tr = ctx.enter_context(tc.tile_pool(name="ps_tr", bufs=1, space="PSUM"))
        ps_mm = ctx.enter_context(tc.tile_pool(name="ps_mm", bufs=2, space="PSUM"))
        ps_pw = ctx.enter_context(tc.tile_pool(name="ps_pw", bufs=1, space="PSUM"))
        ps_tw = ctx.enter_context(tc.tile_pool(name="ps_tw", bufs=1, space="PSUM"))
        ps_ot = ctx.enter_context(tc.tile_pool(name="ps_ot", bufs=1, space="PSUM"))
        ps_o = ctx.enter_context(tc.tile_pool(name="ps_o", bufs=1, space="PSUM"))

        idx0_t = consts.tile([BLK, F], i32)
        nc.sync.dma_start(idx0_t[:], idx0_d.ap())
        idxr_t = consts.tile([BLK, (NBLK_D - 1) * F], i32)
        nc.sync.dma_start(idxr_t[:], idxr_d.ap())
        w1t = consts.tile([128, 4 * HID], bf16)
        nc.sync.dma_start(w1t[:], w1t_d.ap())
        b1t = consts.tile([128, 2], f32)
        nc.sync.dma_start(b1t[:], b1t_d.ap())
        w2t = consts.tile([128, 2 * NCLS], bf16)
        nc.sync.dma_start(w2t[:], w2t_d.ap())
        identb = consts.tile([128, 128], f32)
        nc.sync.dma_start(identb[:], identb_d.ap())
        identb2 = consts.tile([128, 128], bf16)
        nc.sync.dma_start(identb2[:], identb2_d.ap())
        ft_t = consts.tile([BLK, 2 * NBLK_D], f32)
        nc.sync.dma_start(ft_t[:], ft_d.ap())
        iota_t = consts.tile([128, DST_PC], f32)
        nc.sync.dma_start(iota_t[:], iota_d.ap())

        preA = {}
        for pb in (NBLK_D - 2, NBLK_D - 1):
            for layer in range(2):
                pA = consts.tile([128, DST_PC], bf16, tag=f"pA{pb}_{layer}",
                                 name=f"pA{pb}_{layer}")
                nc.vector.tensor_scalar(
                    pA[:], iota_t[:],
                    ft_t[:, 2 * pb + layer:2 * pb + layer + 1],
                    None, mybir.AluOpType.is_equal)
                preA[(pb, layer)] = pA

        # resident h1^T: [hid-half on partitions, slot-cols] bf16
        h1T = [h1pool.tile([128, UB], bf16, tag=f"h1T{hc}", name=f"h1T{hc}")
               for hc in range(2)]
        # persistent out^T accumulators, split by dst range so the low-dst
        # stores can issue before the tail (first-ref dsts are monotone over
        # slots; host verifies blocks >= SPLIT_B only reference dst >= SPLIT_D)
        outT_A = ps_ot.tile([NCLS, SPLIT_D], f32, tag="outT_A", space="PSUM")
        outT_B = ps_ot.tile([NCLS, DST_PC - SPLIT_D], f32, tag="outT_B",
                            space="PSUM")
        A_LAST = NBLK_D - 1 if conservative else SPLIT_B - 1

        def emit_gathers(g, b, k0, k1):
            for k in range(k0, k1):
                if b == 0:
                    iap = idx0_t[:, k:k + 1]
                else:
                    iap = idxr_t[:, (b - 1) * F + k:(b - 1) * F + k + 1]
                nc.gpsimd.indirect_dma_start(
                    out=g[:, k * IN_F:(k + 1) * IN_F], out_offset=None,
                    in_=feat.ap(),
                    in_offset=bass.IndirectOffsetOnAxis(ap=iap, axis=0),
                )

        def emit_block_compute(b, hs):
            """transposes + W1 + relu into h1T for pooled block b."""
            hhT = [hpool.tile([128, BLK], bf16, tag=f"hhT{fc}",
                              name=f"hhT{fc}_{b}") for fc in range(4)]
            for fc in range(4):
                ptr = ps_tr.tile([128, 128], f32, tag="ptr", space="PSUM")
                nc.tensor.transpose(ptr[:], hs[:, fc * 128:(fc + 1) * 128],
                                    identb[:])
                nc.scalar.activation(hhT[fc][:], ptr[:],
                                     mybir.ActivationFunctionType.Copy)
            for hc in range(2):
                pm = ps_mm.tile([128, 512], f32, tag="pm", space="PSUM")
                for fc in range(4):
                    nc.tensor.matmul(
                        pm[:, 0:BLK],
                        lhsT=w1t[:, fc * HID + hc * 128: fc * HID + hc * 128 + 128],
                        rhs=hhT[fc][:],
                        start=(fc == 0), stop=(fc == 3),
                    )
                nc.scalar.activation(h1T[hc][:, b * BLK:(b + 1) * BLK],
                                     pm[:, 0:BLK],
                                     mybir.ActivationFunctionType.Relu,
                                     bias=b1t[:, hc:hc + 1])

        def emit_block(b):
            g = gpool.tile([BLK, F * IN_F], f32, tag="g", name=f"g_{b}")
            emit_gathers(g, b, 0, F)
            v = g[:].rearrange("p (r two f) -> p r two f", two=2, f=IN_F)
            s1 = spool.tile([BLK, 5 * IN_F], f32, tag="s1", name=f"s1_{b}")
            s2 = spool.tile([BLK, 2 * IN_F], f32, tag="s2", name=f"s2_{b}")
            hs = spool.tile([BLK, IN_F], f32, tag="hs", name=f"hs_{b}")
            nc.vector.tensor_add(s1[:], v[:, :, 0, :], v[:, :, 1, :])
            nc.vector.tensor_add(s2[:], s1[:, 0:2 * IN_F],
                                 s1[:, 2 * IN_F:4 * IN_F])
            nc.vector.tensor_add(s2[:, 0:IN_F], s2[:, 0:IN_F],
                                 s2[:, IN_F:2 * IN_F])
            nc.vector.tensor_add(hs[:], s2[:, 0:IN_F], s1[:, 4 * IN_F:5 * IN_F])
            emit_block_compute(b, hs)

        def emit_hop2_cols(c0, W):
            """h1w2 rows for slot-cols [c0, c0+W) and A/SEL matmuls."""
            s = c0 // 512
            pw = ps_pw.tile([NCLS, 512], f32, tag="pw", space="PSUM")
            for hc in range(2):
                nc.tensor.matmul(
                    pw[:, 0:W],
                    lhsT=w2t[:, hc * NCLS:(hc + 1) * NCLS],
                    rhs=h1T[hc][:, c0:c0 + W],
                    start=(hc == 0), stop=(hc == 1),
                )
            pwb = wpool.tile([NCLS, 512], bf16, tag="pwb", name=f"pwb_{s}")
            nc.scalar.activation(pwb[:, 0:W], pw[:, 0:W],
                                 mybir.ActivationFunctionType.Copy)
            for j in range(W // BLK):
                b = c0 // BLK + j
                ptw = ps_tw.tile([128, NCLS], bf16, tag="ptw", space="PSUM")
                nc.tensor.matmul(ptw[:], lhsT=pwb[:, j * BLK:(j + 1) * BLK],
                                 rhs=identb2[0:NCLS, 0:NCLS],
                                 start=True, stop=True, is_transpose=True)
                rws = wpool.tile([128, NCLS], bf16, tag="rws", name=f"rws_{b}")
                nc.scalar.activation(rws[:], ptw[:],
                                     mybir.ActivationFunctionType.Copy)
                for layer in range(2):
                    if (b, layer) in preA:
                        A = preA[(b, layer)]
                    else:
                        A = apool.tile([128, DST_PC], bf16, tag=f"A{layer}",
                                       name=f"A{layer}_{b}")
                        nc.vector.tensor_scalar(
                            A[:], iota_t[:],
                            ft_t[:, 2 * b + layer:2 * b + layer + 1],
                            None, mybir.AluOpType.is_equal)
                    if conservative or b < SPLIT_B:
                        nc.tensor.matmul(
                            outT_A[:], lhsT=rws[:], rhs=A[:, 0:SPLIT_D],
                            start=(b == 0 and layer == 0),
                            stop=(b == A_LAST and layer == 1),
                        )
                    nc.tensor.matmul(
                        outT_B[:], lhsT=rws[:], rhs=A[:, SPLIT_D:DST_PC],
                        start=(b == 0 and layer == 0),
                        stop=(b == NBLK_D - 1 and layer == 1),
                    )

        for b in range(NBLK_D - 2):
            emit_block(b)
            if b % 4 == 3 and 7 <= b <= 35:
                emit_hop2_cols(512 * (b // 4 - 1), 512)

        def emit_out(src_psum, d0, nd):
            obx = opool.tile([NCLS, 512], bf16, tag="ob", name=f"ob_{d0}")
            nc.scalar.activation(obx[:, 0:nd], src_psum[:],
                                 mybir.ActivationFunctionType.Copy)
            for q in range(nd // BLK):
                pf = ps_o.tile([128, NCLS], bf16, tag="pf", space="PSUM")
                nc.tensor.matmul(pf[:], lhsT=obx[:, q * BLK:(q + 1) * BLK],
                                 rhs=identb2[0:NCLS, 0:NCLS],
                                 start=True, stop=True, is_transpose=True)
                ot = opool.tile([128, NCLS], f32, tag="ot",
                                name=f"ot_{d0}_{q}")
                nc.vector.tensor_copy(ot[:], pf[:])
                nc.sync.dma_start(
                    out.ap()[d0 + q * BLK:d0 + (q + 1) * BLK, :], ot[:])

        # last two blocks with early slot-0..7 gathers + stub adds
        gt = {}
        s2a = {}
        for b in (NBLK_D - 2, NBLK_D - 1):
            g = gpool.tile([BLK, F * IN_F], f32, tag=f"g{b}", name=f"g_{b}",
                           bufs=1)
            gt[b] = g
            emit_gathers(g, b, 0, 8)
            v = g[:].rearrange("p (r two f) -> p r two f", two=2, f=IN_F)
            s1 = consts.tile([BLK, 4 * IN_F], f32, tag=f"s{b}a", name=f"s{b}a")
            s2 = consts.tile([BLK, 2 * IN_F], f32, tag=f"s{b}b", name=f"s{b}b")
            nc.vector.tensor_add(s1[:], v[:, 0:4, 0, :], v[:, 0:4, 1, :])
            nc.vector.tensor_add(s2[:], s1[:, 0:2 * IN_F],
                                 s1[:, 2 * IN_F:4 * IN_F])
            nc.vector.tensor_add(s2[:, 0:IN_F], s2[:, 0:IN_F],
                                 s2[:, IN_F:2 * IN_F])
            s2a[b] = s2
        if not conservative:
            emit_out(outT_A, 0, SPLIT_D)   # dsts < SPLIT_D final at block 29
        emit_hop2_cols(512 * 8, 512)   # blocks 32-35, ready at loop end
        for b in (NBLK_D - 2, NBLK_D - 1):
            emit_gathers(gt[b], b, 8, F)
            g = gt[b]
            p89 = consts.tile([BLK, IN_F], f32, tag=f"s{b}c", name=f"s{b}c")
            hs = consts.tile([BLK, IN_F], f32, tag=f"s{b}d", name=f"s{b}d")
            nc.vector.tensor_add(p89[:], g[:, 8 * IN_F:9 * IN_F],
                                 g[:, 9 * IN_F:10 * IN_F])
            nc.vector.tensor_add(hs[:], s2a[b][:, 0:IN_F], p89[:])
            emit_block_compute(b, hs)
            emit_hop2_cols(b * BLK, BLK)

        # final stores
        if conservative:
            emit_out(outT_A, 0, SPLIT_D)
        emit_out(outT_B, SPLIT_D, DST_PC - SPLIT_D)

    return nc


def _get_nc(conservative=False):
    global _BUILT
    if _BUILT is None:
        _BUILT = {}
    if conservative not in _BUILT:
        _BUILT[conservative] = build_nc(conservative)
    return _BUILT[conservative]


def _prep_core_dedup(src0, src1, core):
    """Slot list (<=2 referencing dsts per slot), gather idx + f-tables."""
    s = src1[core * GRP_PC:(core + 1) * GRP_PC].astype(np.int64)
    slots, f1, f2 = [], [], []
    open_slot = {}
    for e in range(GRP_PC):
        g = int(s[e]); d = e // F
        j = open_slot.get(g)
        if j is not None:
            f2[j] = d
            del open_slot[g]        # 3rd ref opens a new slot
        else:
            open_slot[g] = len(slots)
            slots.append(g); f1.append(d); f2.append(SENT)
    ns = len(slots)
    assert ns <= UB, f"core {core}: {ns} slots > {UB}"
    slots += [0] * (UB - ns)
    f1 += [SENT] * (UB - ns)
    f2 += [SENT] * (UB - ns)
    sl = np.asarray(slots, dtype=np.int64)
    G = src0[(sl[:, None] * F + np.arange(F)[None, :])]     # [UB, 10]
    idx = np.ascontiguousarray(
        G.reshape(NBLK_D, BLK, F).transpose(1, 0, 2).reshape(BLK, NBLK_D * F)
    ).astype(np.int32)
    ftab = np.stack([np.asarray(f1), np.asarray(f2)], axis=-1)  # [UB, 2]
    ft = np.ascontiguousarray(
        ftab.reshape(NBLK_D, BLK, 2).transpose(1, 0, 2).reshape(BLK, 2 * NBLK_D)
    ).astype(np.float32)
    return idx, ft


def _prep_weights(W1, b1, W2):
    import ml_dtypes
    w1t = (np.asarray(W1, dtype=np.float32) / np.float32(F)).reshape(
        4, 128, HID).transpose(1, 0, 2).reshape(128, 4 * HID)
    w2t = (np.asarray(W2, dtype=np.float32) / np.float32(F)).reshape(
        2, 128, NCLS).transpose(1, 0, 2).reshape(128, 2 * NCLS)
    b1t = np.asarray(b1, dtype=np.float32).reshape(2, 128).T
    identb = np.eye(128, dtype=np.float32)
    return {
        "w1t": np.ascontiguousarray(w1t).astype(ml_dtypes.bfloat16),
        "w2t": np.ascontiguousarray(w2t).astype(ml_dtypes.bfloat16),
        "b1t": np.ascontiguousarray(b1t),
        "identb": identb,
    }


def _run(inputs, trace=False, trace_kwargs=None):
    from concourse.bass_utils import run_bass_kernel_spmd

    features = np.ascontiguousarray(inputs["features"], dtype=np.float32)
    b2 = np.ascontiguousarray(inputs["b2"], dtype=np.float32)
    src0 = np.asarray(inputs["src0"]).astype(np.int64)
    src1 = np.asarray(inputs["src1"]).astype(np.int64)
    wts = _prep_weights(inputs["W1"], inputs["b1"], inputs["W2"])
    import ml_dtypes
    wts["identb2"] = np.eye(128, dtype=np.float32).astype(ml_dtypes.bfloat16)
    wts["iota"] = np.broadcast_to(np.arange(DST_PC, dtype=np.float32),
                                  (128, DST_PC)).copy()

    in_maps = []
    conservative = False
    for c in range(NC_N):
        idx, ft = _prep_core_dedup(src0, src1, c)
        # blocks >= 30 must only reference dst >= 384 for the split-out build
        tail_f = ft[:, 2 * 30:]
        if (tail_f[tail_f < SENT] < 384).any():
            conservative = True
        in_maps.append({
            "feat": features,
            "idx0": np.ascontiguousarray(idx[:, :F]),
            "idxr": np.ascontiguousarray(idx[:, F:]),
            "ft": ft,
            **wts,
        })
    nc = _get_nc(conservative)
    kw = {}
    if trace:
        kw = {"trace": True, "trace_kwargs": trace_kwargs or {}}
    res = run_bass_kernel_spmd(nc, in_maps, list(range(NC_N)), **kw)
    full = np.concatenate([res.results[c]["out"] for c in range(NC_N)], axis=0)
    full = full + b2[None, :]
    return full, res


def kernel(features, W1, b1, W2, b2, src0, dst0, src1, dst1):
    ins = dict(features=features, W1=W1, b1=b1, W2=W2, b2=b2,
               src0=src0, dst0=dst0, src1=src1, dst1=dst1)
    d0 = np.asarray(dst0); d1 = np.asarray(dst1)
    fixed = (d0 == np.arange(N1 * F) // F).all() and \
            (d1 == np.arange(N2 * F) // F).all()
    if not fixed:
        # general (unexpected) dst pattern: numpy fallback for correctness
        f = np.asarray(features, dtype=np.float64)
        m = f[np.asarray(src0)]
        s = np.zeros((N1, IN_F)); np.add.at(s, d0, m)
        deg = np.bincount(d0, minlength=N1).clip(1)
        h = np.maximum(s / deg[:, None] @ np.asarray(W1) + np.asarray(b1), 0)
        m = h[np.asarray(src1)]
        s = np.zeros((N2, HID)); np.add.at(s, d1, m)
        deg = np.bincount(d1, minlength=N2).clip(1)
        return ((s / deg[:, None]) @ np.asarray(W2) + np.asarray(b2)
                ).astype(np.float32)
    out, _ = _run(ins)
    return out
